# revision 14
# baseline (speedup 1.0000x reference)
"""GCN encoder (2x GCNConv + ReLU + global mean pool) as a Bass SPMD kernel
for 8 trn2 NeuronCores.

Formulation (per layer, A includes self loops, D = degree over dest):
    out = D^-1/2 A D^-1/2 (x W + b)   with b == 0 enforced
        = dinv * (AGG @ W)            AGG[n] = sum_{e: row=n} T[col_e],
                                      T = dinv * x   (layer input scaled)
Layer 1: T2 = dinv * relu(out1) = dinv^2 * relu(AGG1 @ W1)
Layer 2: out2 = dinv * (AGG2 @ W2); pooled = segsum(out2, batch) / cnt

Distribution: nodes block-sharded over 8 cores; each core aggregates its own
destination rows. Both layers' scaled-feature tables use the same shard-concat
[core, slot] layout, AllGathered once per layer, so one shared index/selection
structure serves both sweeps. Gather windows are core-pairs (25088 rows, int16
range); gathers for consecutive (span, window) calls sit on the 4 SWDGE queues
whose descriptor generation runs on disjoint Q7 core pairs, so prefetching 3
spans ahead keeps all 8 Q7 cores generating descriptors concurrently.

Edges are grouped by (destination block-pair, source core-pair): a [128,256]
slice of a full-PSUM-bank f32 tile accumulates each pair's aggregate as
matmuls of gathered source rows against one-hot selection matrices that are
PRECOMPUTED ON HOST and streamed from DRAM (512B/partition per piece) — no
per-piece vector-engine work. Self-loop edges skip the gather via an identity
matmul on resident local rows; global mean pool uses per-block one-hot
matrices built on DVE from graph ids mod 128 (each core spans <128 ids), with
host masks unfolding the mod before the AllReduce.

SPMD constraint: one program runs on all 8 cores; group capacities are max'ed
across cores, surplus idx slots point at row 0 with an all-zero selection
column. A per-core greedy node->slot permutation flattens group sizes.
"""
import numpy as np
import ml_dtypes

import concourse.bass as bass
import concourse.mybir as mybir
import concourse.tile as tile
from concourse import bacc

P = 128
NCORE = 8
bf16 = mybir.dt.bfloat16
f32 = mybir.dt.float32
i16 = mybir.dt.int16


class Cfg:
    def __init__(self, n_nodes, n_graphs):
        assert n_nodes % NCORE == 0
        self.N = n_nodes
        self.G = n_graphs
        self.n_sh = n_nodes // NCORE          # owned nodes per core (12500)
        self.nblk = -(-self.n_sh // P)        # blocks per core (98)
        self.n_shp = self.nblk * P            # padded shard rows (12544)
        self.nwin = NCORE // 2                # gather windows = core pairs
        self.tab_rows = 2 * self.n_shp        # rows per window (25088)
        assert self.tab_rows <= 32000         # int16 gather indices
        self.nt_full = NCORE * self.n_shp
        self.npair = self.nblk // 2           # dst block-pairs (49)
        self.span = 4                         # pairs per gather call
        self.nspan = -(-self.npair // self.span)
        assert self.G <= 2 * P


def host_prep(cfg, edge_index, batch):
    N, G = cfg.N, cfg.G
    row = np.asarray(edge_index[0], dtype=np.int64)
    col = np.asarray(edge_index[1], dtype=np.int64)
    # degree over col including self loops
    deg = np.bincount(col, minlength=N).astype(np.float32) + 1.0

    core_of = row // cfg.n_sh
    src_core = col // cfg.n_sh
    w_of = src_core // 2                      # source window (permutation-inv)

    # --- per-core greedy node->slot permutation: flatten per-(block, window)
    # in-edge counts so the cross-core capacity max is tight.
    d4 = np.zeros((N, cfg.nwin), dtype=np.int32)
    np.add.at(d4, (row, w_of), 1)

    perm = np.full((NCORE, cfg.n_shp), -1, dtype=np.int64)   # slot -> local node
    inv = np.zeros((NCORE, cfg.n_sh), dtype=np.int64)        # local node -> slot
    for c in range(NCORE):
        d = d4[c * cfg.n_sh:(c + 1) * cfg.n_sh].astype(np.float64)
        order_n = np.argsort(-d.sum(1), kind="stable")
        target = d.sum(0) / cfg.nblk + 1e-9
        sums = np.zeros((cfg.nblk, cfg.nwin))
        fill = np.zeros(cfg.nblk, dtype=np.int64)
        for n in order_n:
            score = ((sums + d[n]) / target).max(axis=1)
            score[fill >= P] = np.inf
            b = int(np.argmin(score))
            sums[b] += d[n]
            perm[c, b * P + fill[b]] = n
            inv[c, n] = b * P + fill[b]
            fill[b] += 1

    r_slot = inv[core_of, row % cfg.n_sh]
    pair = r_slot // (2 * P)
    rl = r_slot % (2 * P)                              # 0..255 within pair
    s_slot = inv[src_core, col % cfg.n_sh]
    tab_off = (src_core % 2) * cfg.n_shp + s_slot      # within-window offset

    # --- shared (both layers) call/piece structure ---
    order = np.lexsort((tab_off, w_of, pair, core_of))
    core_s = core_of[order]
    pair_s = pair[order]
    w_s = w_of[order]
    rl_s = rl[order]
    off_s = tab_off[order]

    sizes = np.zeros((NCORE, cfg.npair, cfg.nwin), dtype=np.int64)
    np.add.at(sizes, (core_s, pair_s, w_s), 1)
    caps16 = ((sizes.max(axis=0) + 15) // 16) * 16     # [npair, nwin]

    grp_start = np.zeros((NCORE, cfg.npair, cfg.nwin), dtype=np.int64)
    grp_start.reshape(-1)[1:] = np.cumsum(sizes.reshape(-1))[:-1]

    calls = []
    icol = 0    # idx tile column cursor (16 idxs per column)
    pcol = 0    # selection tile cursor (one 256-col selection per piece)
    for s in range(cfg.nspan):
        pairs = range(s * cfg.span, min((s + 1) * cfg.span, cfg.npair))
        for w in range(cfg.nwin):
            cap = int(sum(caps16[p, w] for p in pairs))
            cap128 = -(-cap // P) * P                  # pad call to full cols
            ncol = cap128 // P
            if ncol == 0:
                continue
            pieces = []
            groups = []
            off = 0
            for p in pairs:
                c16 = int(caps16[p, w])
                if c16 == 0:
                    continue
                groups.append((p, off, c16))
                pos = off
                while pos < off + c16:
                    coli = pos // P
                    p0 = pos % P
                    take = min(P - p0, off + c16 - pos)
                    pieces.append([p, coli, p0, p0 + take, pcol])
                    pcol += 1
                    pos += take
                off += c16
            calls.append(dict(span=s, w=w, cap=cap128, icol=icol, ncol=ncol,
                              pieces=pieces, groups=groups))
            icol += cap128 // 16
    icols, pcols = icol, pcol

    # span -> (first piece idx, piece count) for per-span selection DMA
    span_pieces = []
    pc_cursor = 0
    for s in range(cfg.nspan):
        n = sum(len(c["pieces"]) for c in calls if c["span"] == s)
        span_pieces.append((pc_cursor, n))
        pc_cursor += n

    idx_all = np.zeros((NCORE, 16, icols), dtype=np.int16)
    rl_all = np.full((NCORE, P, pcols), -1, dtype=np.int64)
    for call in calls:
        w = call["w"]
        for pc in call["pieces"]:
            p, coli, p0, p1, pci = pc
            slot_off = next(so for pp, so, cc in call["groups"] if pp == p)
            for c in range(NCORE):
                n = int(sizes[c, p, w])
                s0 = grp_start[c, p, w]
                g_lo = coli * P + p0 - slot_off
                g_hi = coli * P + p1 - slot_off
                lo, hi = max(g_lo, 0), min(g_hi, n)
                if lo < hi:
                    rl_all[c][p0 + (lo - g_lo):p0 + (hi - g_lo), pci] = \
                        rl_s[s0 + lo:s0 + hi]
        for p, slot_off, c16 in call["groups"]:
            base = call["icol"] * 16 + slot_off
            for c in range(NCORE):
                n = int(sizes[c, p, w])
                s0 = grp_start[c, p, w]
                if n:
                    pos = base + np.arange(n)
                    idx_all[c][pos % 16, pos // 16] = off_s[s0:s0 + n].astype(np.int16)
                # pad slots stay 0 (row 0 of window), zero selection column

    # host-built one-hot selection tiles: [P, pcols * 256] bf16 per core
    sel_tiles = []
    for c in range(NCORE):
        sel = np.zeros((P, pcols, 2 * P), dtype=ml_dtypes.bfloat16)
        pp, cc = np.nonzero(rl_all[c] >= 0)
        sel[pp, cc, rl_all[c][pp, cc]] = 1.0
        sel_tiles.append(np.ascontiguousarray(sel.reshape(P, pcols * 2 * P)))

    struct = dict(
        calls=calls, icols=icols, pcols=pcols, span_pieces=span_pieces,
        idx_tiles=[np.tile(idx_all[c], (8, 1)) for c in range(NCORE)],
        sel_tiles=sel_tiles)

    # --- per-core aux tables ---
    batch = np.asarray(batch, dtype=np.int64)
    cnts = np.bincount(batch, minlength=G).astype(np.float32)
    deg_t, bt, masks = [], [], []
    for c in range(NCORE):
        pc = perm[c]
        valid = pc >= 0
        d = np.ones(cfg.n_shp, dtype=np.float32)
        d[valid] = deg[c * cfg.n_sh + pc[valid]]
        deg_t.append(np.ascontiguousarray(d.reshape(cfg.nblk, P).T))
        b = np.full(cfg.n_shp, -1.0, dtype=np.float32)
        gids = batch[c * cfg.n_sh + pc[valid]]
        assert gids.max() - gids.min() < P, "core spans >=128 graph ids"
        b[valid] = gids % P
        bt.append(np.ascontiguousarray(
            b.reshape(cfg.nblk, P).T).astype(ml_dtypes.bfloat16))
        m = np.zeros((P, 2), dtype=np.float32)
        m[gids[gids < P] % P, 0] = 1.0
        m[gids[gids >= P] % P, 1] = 1.0
        masks.append(m)

    inv_pad = np.zeros(2 * P, dtype=np.float32)
    inv_pad[:G] = 1.0 / np.maximum(cnts, 1.0)
    inv_tile = np.ascontiguousarray(inv_pad.reshape(2, P).T)  # [128, 2]

    return dict(struct=struct, deg_t=deg_t, batch_t=bt, masks=masks,
                inv_tile=inv_tile, perm=perm)


def build_program(cfg, prep):
    nc = bacc.Bacc("TRN2", target_bir_lowering=False, num_devices=NCORE,
                   num_swdge_queues=4)
    st = prep["struct"]
    calls = st["calls"]
    nblk, npair, nspan = cfg.nblk, cfg.npair, cfg.nspan

    x_in = nc.declare_dram_parameter("x_local", [cfg.n_shp, P], f32, isOutput=False)
    w1_in = nc.declare_dram_parameter("w1", [P, P], f32, isOutput=False)
    w2_in = nc.declare_dram_parameter("w2", [P, P], f32, isOutput=False)
    deg_in = nc.declare_dram_parameter("deg_t", [P, nblk], f32, isOutput=False)
    iota_in = nc.declare_dram_parameter("iota", [P, P], bf16, isOutput=False)
    ident_in = nc.declare_dram_parameter("ident", [P, P], bf16, isOutput=False)
    idx_in = nc.declare_dram_parameter("idx", [P, st["icols"]], i16, isOutput=False)
    sel_in = nc.declare_dram_parameter("sel", [P, st["pcols"] * 2 * P], bf16,
                                       isOutput=False)
    batch_in = nc.declare_dram_parameter("batch_t", [P, nblk], bf16, isOutput=False)
    mask_in = nc.declare_dram_parameter("masks", [P, 2], f32, isOutput=False)
    invc_in = nc.declare_dram_parameter("inv_cnt", [P, 2], f32, isOutput=False)
    out_ext = nc.declare_dram_parameter("out", [2 * P, P], f32, isOutput=True)

    t1_shard = nc.dram_tensor("t1_shard", [cfg.n_shp, P], bf16)
    t1_full = nc.dram_tensor("t1_full", [cfg.nt_full, P], bf16, addr_space="Shared")
    t2_shard = nc.dram_tensor("t2_shard", [cfg.n_shp, P], bf16)
    t2_full = nc.dram_tensor("t2_full", [cfg.nt_full, P], bf16, addr_space="Shared")
    pool_part = nc.dram_tensor("pool_part", [2 * P, P], f32)
    pool_full = nc.dram_tensor("pool_full", [2 * P, P], f32, addr_space="Shared")

    max_ncol = max(c["ncol"] for c in calls)
    max_span_pc = max(n for _, n in st["span_pieces"])

    with tile.TileContext(nc) as tc:
        with tc.tile_pool(name="const", bufs=1) as cpool, \
             tc.tile_pool(name="xio", bufs=3) as xpool, \
             tc.tile_pool(name="tsb", bufs=1) as tpool, \
             tc.tile_pool(name="gath", bufs=12) as gpool, \
             tc.tile_pool(name="sel", bufs=2) as spool, \
             tc.tile_pool(name="blk", bufs=4) as bpool, \
             tc.tile_pool(name="agg", bufs=3, space="PSUM") as apool, \
             tc.tile_pool(name="hp", bufs=3, space="PSUM") as hpool, \
             tc.tile_pool(name="pool", bufs=1, space="PSUM") as ppool:

            # ---- constants ----
            iota = cpool.tile([P, P], bf16)
            nc.sync.dma_start(out=iota[:], in_=iota_in[:])
            ident = cpool.tile([P, P], bf16)
            nc.sync.dma_start(out=ident[:], in_=ident_in[:])
            idx_sb = cpool.tile([P, st["icols"]], i16)
            nc.sync.dma_start(out=idx_sb[:], in_=idx_in[:])
            batch_sb = cpool.tile([P, nblk], bf16)
            nc.sync.dma_start(out=batch_sb[:], in_=batch_in[:])
            mask_sb = cpool.tile([P, 2], f32)
            nc.sync.dma_start(out=mask_sb[:], in_=mask_in[:])
            invc_sb = cpool.tile([P, 2], f32)
            nc.sync.dma_start(out=invc_sb[:], in_=invc_in[:])

            w1f = cpool.tile([P, P], f32)
            nc.sync.dma_start(out=w1f[:], in_=w1_in[:])
            w1_sb = cpool.tile([P, P], bf16)
            nc.vector.tensor_copy(out=w1_sb[:], in_=w1f[:])
            w2f = cpool.tile([P, P], f32)
            nc.sync.dma_start(out=w2f[:], in_=w2_in[:])
            w2_sb = cpool.tile([P, P], bf16)
            nc.vector.tensor_copy(out=w2_sb[:], in_=w2f[:])

            zsel = cpool.tile([P, 2 * P], bf16)
            nc.vector.memset(zsel[:], 0.0)

            degf = cpool.tile([P, nblk], f32)
            nc.sync.dma_start(out=degf[:], in_=deg_in[:])
            sq = cpool.tile([P, nblk], f32)
            nc.scalar.sqrt(out=sq[:], in_=degf[:])
            dinv = cpool.tile([P, nblk], f32)
            nc.vector.reciprocal(out=dinv[:], in_=sq[:])
            dinv2 = cpool.tile([P, nblk], f32)
            nc.vector.tensor_mul(out=dinv2[:], in0=dinv[:], in1=dinv[:])

            # resident scaled tables (self-loop stationaries, AG staging)
            t1_sb = tpool.tile([P, nblk, P], bf16)
            t2_sb = tpool.tile([P, nblk, P], bf16)

            x_r = x_in.rearrange("(nb p) f -> p nb f", p=P)
            t1_r = t1_shard.rearrange("(nb p) f -> p nb f", p=P)
            t2_r = t2_shard.rearrange("(nb p) f -> p nb f", p=P)
            sel_r = sel_in.rearrange("p (pc w) -> p pc w", w=2 * P)

            # ---- prologue: T1 = dinv * x ----
            slab = 7
            for s0 in range(0, nblk, slab):
                xb = xpool.tile([P, slab, P], f32, tag="xb")
                nc.sync.dma_start(out=xb[:], in_=x_r[:, s0:s0 + slab, :])
                for j in range(slab):
                    nc.vector.tensor_scalar(
                        out=t1_sb[:, s0 + j, :], in0=xb[:, j, :],
                        scalar1=dinv[:, s0 + j:s0 + j + 1], scalar2=None,
                        op0=mybir.AluOpType.mult)
            nc.scalar.dma_start(out=t1_r[:], in_=t1_sb[:])
            nc.gpsimd.collective_compute(
                "AllGather", mybir.AluOpType.bypass,
                replica_groups=[list(range(NCORE))],
                ins=[t1_shard[:]], outs=[t1_full[:]])

            pool_bank = ppool.tile([P, 4 * P], f32, space="PSUM")
            pool_ps = pool_bank[:, :P]

            # round-robin engines for per-span selection DMAs
            sel_engines = [nc.sync, nc.scalar]

            def fire_span(span_i, t_full_d, state):
                if span_i >= nspan:
                    return
                for call in calls:
                    if call["span"] != span_i:
                        continue
                    w = call["w"]
                    g_sb = gpool.tile([P, max_ncol, P], bf16, tag="g")
                    nc.gpsimd.dma_gather(
                        g_sb[:, :call["ncol"], :],
                        t_full_d[w * cfg.tab_rows:(w + 1) * cfg.tab_rows, :],
                        idx_sb[:, call["icol"]:call["icol"] + call["cap"] // 16],
                        call["cap"], call["cap"], P,
                        single_packet=False, queue_num=w % 4)
                    state[("g", span_i, w)] = g_sb
                pc0, npc = st["span_pieces"][span_i]
                if npc:
                    s_sb = spool.tile([P, max_span_pc, 2 * P], bf16, tag="sel")
                    sel_engines[span_i % len(sel_engines)].dma_start(
                        out=s_sb[:, :npc, :], in_=sel_r[:, pc0:pc0 + npc, :])
                    state[("s", span_i)] = (s_sb, pc0)

            def sweep(layer, t_full_d, t_loc, w_sb):
                pair_pieces = [[] for _ in range(npair)]
                for call in calls:
                    for p, coli, p0, p1, pci in call["pieces"]:
                        pair_pieces[p].append((call["span"], call["w"], coli, pci))

                state = {}
                fire_span(0, t_full_d, state)
                fire_span(1, t_full_d, state)
                fire_span(2, t_full_d, state)
                for s in range(nspan):
                    pairs = range(s * cfg.span, min((s + 1) * cfg.span, npair))
                    for p in pairs:
                        bank = apool.tile([P, 4 * P], f32, tag="agg",
                                          space="PSUM")
                        ap = bank[:, :2 * P]
                        for h in range(2):
                            b = 2 * p + h
                            # start=True zeroes the whole 2KB bank: only once
                            nc.tensor.matmul(
                                ap[:, h * P:(h + 1) * P],
                                lhsT=t_loc[:, b, :], rhs=ident[:],
                                start=(h == 0), stop=False,
                                skip_group_check=True)
                        npc = len(pair_pieces[p])
                        for i, (sp, w, coli, pci) in enumerate(pair_pieces[p]):
                            g_sb = state[("g", sp, w)]
                            s_sb, pc0 = state[("s", sp)]
                            nc.tensor.matmul(
                                ap[:], lhsT=g_sb[:, coli, :],
                                rhs=s_sb[:, pci - pc0, :],
                                start=False, stop=(i == npc - 1),
                                skip_group_check=True)
                        if npc == 0:
                            nc.tensor.matmul(
                                ap[:], lhsT=t_loc[:, 2 * p, :], rhs=zsel[:],
                                start=False, stop=True, skip_group_check=True)
                        # finalize pair
                        aggT = bpool.tile([P, 2 * P], bf16, tag="aggT")
                        nc.scalar.copy(out=aggT[:], in_=ap[:])
                        for h in range(2):
                            b = 2 * p + h
                            hp = hpool.tile([P, P], f32, tag="h", space="PSUM")
                            nc.tensor.matmul(
                                hp[:], lhsT=aggT[:, h * P:(h + 1) * P],
                                rhs=w_sb[:], start=True, stop=True,
                                skip_group_check=True)
                            if layer == 1:
                                nc.scalar.activation(
                                    out=t2_sb[:, b, :], in_=hp[:],
                                    func=mybir.ActivationFunctionType.Relu,
                                    scale=dinv2[:, b:b + 1])
                            else:
                                o2 = bpool.tile([P, P], bf16, tag="o2")
                                nc.scalar.activation(
                                    out=o2[:], in_=hp[:],
                                    func=mybir.ActivationFunctionType.Copy,
                                    scale=dinv[:, b:b + 1])
                                pm = bpool.tile([P, P], bf16, tag="pm")
                                nc.vector.tensor_tensor(
                                    out=pm[:], in0=iota[:],
                                    in1=batch_sb[:, b:b + 1].to_broadcast([P, P]),
                                    op=mybir.AluOpType.is_equal)
                                nc.tensor.matmul(
                                    pool_ps[:], lhsT=pm[:], rhs=o2[:],
                                    start=(b == 0), stop=(b == nblk - 1),
                                    skip_group_check=True)
                    if layer == 1:
                        # write this span's t2 blocks out for the AllGather
                        b0 = s * cfg.span * 2
                        b1 = min((s + 1) * cfg.span * 2, nblk)
                        nc.scalar.dma_start(out=t2_r[:, b0:b1, :],
                                            in_=t2_sb[:, b0:b1, :])
                    fire_span(s + 3, t_full_d, state)
                if layer == 1:
                    nc.gpsimd.collective_compute(
                        "AllGather", mybir.AluOpType.bypass,
                        replica_groups=[list(range(NCORE))],
                        ins=[t2_shard[:]], outs=[t2_full[:]])

            sweep(1, t1_full, t1_sb, w1_sb)
            sweep(2, t2_full, t2_sb, w2_sb)

            # ---- pool partials -> AllReduce -> divide ----
            for j in range(2):
                ps = xpool.tile([P, P], f32, tag="ps")
                nc.vector.tensor_scalar(
                    out=ps[:], in0=pool_ps[:],
                    scalar1=mask_sb[:, j:j + 1], scalar2=None,
                    op0=mybir.AluOpType.mult)
                nc.sync.dma_start(out=pool_part[j * P:(j + 1) * P, :], in_=ps[:])
            nc.gpsimd.collective_compute(
                "AllReduce", mybir.AluOpType.add,
                replica_groups=[list(range(NCORE))],
                ins=[pool_part[:]], outs=[pool_full[:]])
            for j in range(2):
                pf = xpool.tile([P, P], f32, tag="pf")
                nc.sync.dma_start(out=pf[:], in_=pool_full[j * P:(j + 1) * P, :])
                of = xpool.tile([P, P], f32, tag="of")
                nc.vector.tensor_scalar(
                    out=of[:], in0=pf[:],
                    scalar1=invc_sb[:, j:j + 1], scalar2=None,
                    op0=mybir.AluOpType.mult)
                nc.sync.dma_start(out=out_ext[j * P:(j + 1) * P, :], in_=of[:])

    nc.compile()
    return nc


def make_in_maps(cfg, prep, x, W1, W2):
    x = np.asarray(x, dtype=np.float32)
    iota = np.broadcast_to(np.arange(P, dtype=np.float32), (P, P))
    ident = np.eye(P, dtype=np.float32)
    in_maps = []
    for c in range(NCORE):
        pc = prep["perm"][c]
        valid = pc >= 0
        xl = np.zeros((cfg.n_shp, P), dtype=np.float32)
        xl[valid] = x[c * cfg.n_sh + pc[valid]]
        in_maps.append({
            "x_local": xl,
            "w1": np.asarray(W1, dtype=np.float32),
            "w2": np.asarray(W2, dtype=np.float32),
            "deg_t": prep["deg_t"][c],
            "iota": np.ascontiguousarray(iota).astype(ml_dtypes.bfloat16),
            "ident": ident.astype(ml_dtypes.bfloat16),
            "idx": prep["struct"]["idx_tiles"][c],
            "sel": prep["struct"]["sel_tiles"][c],
            "batch_t": prep["batch_t"][c],
            "masks": prep["masks"][c],
            "inv_cnt": prep["inv_tile"],
        })
    return in_maps


def run(x, edge_index, batch, num_graphs, W1, b1, W2, b2, trace=False):
    from concourse.bass_utils import run_bass_kernel_spmd
    N = int(x.shape[0])
    G = int(num_graphs)
    assert not np.any(np.asarray(b1)) and not np.any(np.asarray(b2)), \
        "nonzero bias not supported"
    cfg = Cfg(N, G)
    prep = host_prep(cfg, np.asarray(edge_index), np.asarray(batch))
    nc = build_program(cfg, prep)
    in_maps = make_in_maps(cfg, prep, x, W1, W2)
    res = run_bass_kernel_spmd(nc, in_maps, list(range(NCORE)), trace=trace)
    out = res.results[0]["out"][:G].astype(np.float32)
    return out, res


def kernel(x, edge_index, batch, num_graphs, W1, b1, W2, b2):
    """Full-input entry point: takes the unsharded problem, distributes it
    across 8 NeuronCores internally, returns the pooled [num_graphs, 128]
    float32 output."""
    out, _ = run(np.asarray(x), np.asarray(edge_index), np.asarray(batch),
                 int(num_graphs), np.asarray(W1), b1, np.asarray(W2), b2)
    return out
